# revision 1
# baseline (speedup 1.0000x reference)
"""Multi-head causal self-attention on 8 TRN2 NeuronCores.

Sharding: data parallel over batch (2) x tensor parallel over heads (16 -> 4
groups of 4 heads).  Core c handles batch c//4 and heads 4*(c%4) .. 4*(c%4)+3.
Each core computes a partial output-projection (its 4 heads' contribution,
[S, D]); the host sums the 4 partials per batch and adds the biases.
No device collectives needed.

Per-core device program (all matmul compute in bf16, f32 PSUM accumulate):
  P1: QT,KT = (x @ WqT, x @ WkT) produced transposed [e, s]; V produced
      natural [s, e] with a ones column appended per head (softmax
      denominators fall out of the AV matmul).  The first i-block's score
      tiles + exps are emitted between K and V so ACT (otherwise idle in
      P1) builds an exp reservoir before phase 2 starts.
  P2/P3 software-pipelined one group deep: group g = (i-block, head-pair).
      Emission order per slot: scoresT+exp for group g, then AV (+ O
      transpose + Wo projection) for group g-1.  The Tile scheduler then
      uses the fully-exp'd previous group as filler work for the PE while
      the current group's scores wait on ACT, instead of idling.
      O s-tiles are transposed by a regular matmul against the identity
      (moving operand) -- ~4x cheaper than PE transpose-mode, and exact.

Host folds: 1/sqrt(dk) into Wq/bq; V-bias contribution = wo @ bv (rows of a
softmax sum to exactly 1) and bo are added on the host.  Output partials are
bf16 (summed in f32 on the host).
"""

import numpy as np
import ml_dtypes
from contextlib import ExitStack

import concourse.bass as bass
import concourse.mybir as mybir
import concourse.tile as tile
from concourse import bacc
from concourse.bass_utils import run_bass_kernel_spmd
from concourse.masks import make_upper_triangular, make_identity

BF16 = ml_dtypes.bfloat16
F32 = mybir.dt.float32
BF = mybir.dt.bfloat16

B = 2
S = 2048
D = 2048
H = 16
DK = 128
NCORES = 8
HPC = 4                  # heads per core
E = HPC * DK             # 512 = output cols per core for q/k/v
P = 128
NDC = D // P             # 16 d-chunks
NST = S // P             # 16 s-tiles
NSB = S // 512           # 4 s/i blocks of 512
DKP = DK + 1             # dk + ones column
N_WARMUP = 44            # dummy matmuls to warm the PE HAM during DMA ramp


def _build_nc():
    nc = bacc.Bacc("TRN2", target_bir_lowering=False, debug=False)

    xt = nc.dram_tensor("xt", [D, S], BF, kind="ExternalInput").ap()
    wqt = nc.dram_tensor("wqt", [D, E], BF, kind="ExternalInput").ap()
    wkt = nc.dram_tensor("wkt", [D, E], BF, kind="ExternalInput").ap()
    wvt = nc.dram_tensor("wvt", [D, E], BF, kind="ExternalInput").ap()
    wot = nc.dram_tensor("wot", [E, D], BF, kind="ExternalInput").ap()
    bqd = nc.dram_tensor("bq", [E], F32, kind="ExternalInput").ap()
    bkd = nc.dram_tensor("bk", [E], F32, kind="ExternalInput").ap()
    outd = nc.dram_tensor("out", [S, D], BF, kind="ExternalOutput").ap()

    with tile.TileContext(nc) as tc, ExitStack() as ctx:
        # PSUM: sc 2x[128,1024] (4 banks) + acc 4x[128,512] (4) = 8
        pst = ctx.enter_context(tc.tile_pool(name="pst", bufs=2, space="PSUM"))
        persist = ctx.enter_context(tc.tile_pool(name="persist", bufs=1))

        qt_t = [persist.tile([P, S], BF, name=f"qt{h}", tag=f"qt{h}") for h in range(HPC)]
        kt_t = [persist.tile([P, S], BF, name=f"kt{h}", tag=f"kt{h}") for h in range(HPC)]
        v_t = [persist.tile([P, HPC, DKP], BF, name=f"v{j}", tag=f"v{j}") for j in range(NST)]
        tri = persist.tile([P, P], BF, name="tri", tag="tri")
        ident = persist.tile([P, P], BF, name="ident", tag="ident")
        bq_sb = persist.tile([P, HPC], F32, name="bq_sb", tag="bq_sb")
        bk_sb = persist.tile([P, HPC], F32, name="bk_sb", tag="bk_sb")
        # i-block 0/1 exp tiles live in the persistent pool: they are written
        # during P1 (between K and V) before the phase-2 exp ring opens.
        e0_t = [[persist.tile([P, 1024], BF, name=f"e0_{g}_{j}", tag=f"e0_{g}_{j}")
                 for j in range(4)] for g in range(2)]
        e1_t = [[persist.tile([P, 1024], BF, name=f"e1_{g}_{j}", tag=f"e1_{g}_{j}")
                 for j in range(8)] for g in range(2)]

        # PE warmup during the input-DMA ramp (results are never read); the
        # operand is produced by a single fast DVE memset, not gpsimd.
        wupd = persist.tile([P, P], BF, name="wupd", tag="wupd")
        nc.vector.memset(wupd[:], 0.0)
        for i in range(N_WARMUP):
            pw = pst.tile([P, 512], F32, name="pw", tag="acc", bufs=4)
            nc.tensor.matmul(pw[:, 0:P], wupd[:], wupd[:], start=True, stop=True)
        # preload the ACT Exp function table now, off the first-score path
        dexp = persist.tile([P, 1], F32, name="dexp", tag="dexp")
        nc.scalar.activation(dexp[:], wupd[:, 0:1],
                             mybir.ActivationFunctionType.Exp)

        # tri[p, f] = 1.0 iff p <= f  (keep j <= i on the diagonal block)
        make_upper_triangular(nc, tri[:], val=1.0, diag=True)
        make_identity(nc, ident[:])
        nc.sync.dma_start(bq_sb[:], bqd.rearrange("(o p) -> p o", p=P))
        nc.sync.dma_start(bk_sb[:], bkd.rearrange("(o p) -> p o", p=P))
        for j in range(NST):
            nc.vector.memset(v_t[j][:, :, DK:DKP], 1.0)

        def sc_group(ib, hp, et_alloc):
            """ScoresT + exp for head-pair hp of i-block ib; returns exp tiles."""
            njt = 4 * ib + 4
            etiles = []
            for jt in range(njt):
                pss = pst.tile([P, 1024], F32, name="pss", tag="sc", bufs=2)
                # band tiles only need i >= jt*128: slice N accordingly
                c0 = max(0, (jt - 4 * ib)) * P
                for k in range(2):
                    h = 2 * hp + k
                    nc.tensor.matmul(
                        pss[:, k * 512 + c0:(k + 1) * 512],
                        kt_t[h][:, jt * P:(jt + 1) * P],
                        qt_t[h][:, ib * 512 + c0:(ib + 1) * 512],
                        start=True, stop=True)
                et_t = et_alloc(jt)
                if jt <= 4 * ib:
                    # full tile written by the matmuls above: one 2D exp
                    nc.scalar.activation(
                        et_t[:], pss[:], mybir.ActivationFunctionType.Exp)
                else:
                    # diag tile: exp only the written per-head regions
                    # (strided 3D AP keeps the read inside this tenant's
                    # writes -- avoids stale-PSUM reads)
                    s_off = jt - 4 * ib
                    et3 = et_t[:].rearrange("p (h w) -> p h w", h=2)
                    ps3 = pss[:].rearrange("p (h w) -> p h w", h=2)
                    nc.scalar.activation(
                        et3[:, :, s_off * P:512], ps3[:, :, s_off * P:512],
                        mybir.ActivationFunctionType.Exp)
                if jt >= 4 * ib:
                    s_off = jt - 4 * ib
                    # zero the diag-masked part of both heads at once
                    et3 = et_t[:].rearrange("p (h w) -> p h w", h=2)
                    nc.vector.tensor_tensor(
                        et3[:, :, s_off * P:(s_off + 1) * P],
                        et3[:, :, s_off * P:(s_off + 1) * P],
                        tri[:, None, :].to_broadcast([P, 2, P]),
                        mybir.AluOpType.mult)
                etiles.append(et_t)
            return etiles

        group_etiles = {}

        # ------------------------------------------------------------------
        # Phase 1: QT/KT [e, s] and V [s, e]; i-block 0 scores between K and V
        # ------------------------------------------------------------------
        with tc.tile_pool(name="pxv", bufs=1) as pxv:
            xt_t = [pxv.tile([P, S], BF, name=f"xt{dc}", tag=f"xt{dc}") for dc in range(NDC)]
            wv_t = [pxv.tile([P, E], BF, name=f"wv{dc}", tag=f"wv{dc}") for dc in range(NDC)]
            def qk_evac(dest, et, psA, psB, bias_sb):
                nc.vector.tensor_scalar_add(
                    dest[et][:, 0:1024], psA[:], bias_sb[:, et:et + 1])
                for i in range(2):
                    nc.vector.tensor_scalar_add(
                        dest[et][:, 1024 + i * 512:1024 + (i + 1) * 512],
                        psB[i][:], bias_sb[:, et:et + 1])

            def qk_psums(et):
                psA = pst.tile([P, 1024], F32, name=f"psA{et}", tag="sc", bufs=2)
                psB = [pst.tile([P, 512], F32, name=f"psB{et}_{i}", tag="acc",
                                bufs=4) for i in range(2)]
                outs = [psA[:, 0:512], psA[:, 512:1024], psB[0][:], psB[1][:]]
                return psA, psB, outs

            # DMA issue plan: the Q pass over e-tile pair 0 consumes xt at
            # ~300 GB/s -- close to the 358 GB/s HBM limit -- so only the
            # bytes pass 0 needs (wq cols 0:256 + xt) are issued first;
            # wq's second half, wk and wv follow after their consumers'
            # predecessors are emitted.  xt goes in column-chunks (a whole
            # [128,2048] transfer runs at single-queue BW ~1/16 aggregate =
            # ~23us, finishing far too late); issue alternates between the
            # two HWDGE engines (SP + ACT, idle here) since each dma_start
            # costs ~0.7us of serial issue time on its engine.
            dma_eng = [nc.sync, nc.scalar]
            with tc.tile_pool(name="pq", bufs=1) as pq:
                wq_t = [pq.tile([P, E], BF, name=f"wq{dc}", tag=f"wq{dc}") for dc in range(NDC)]
                si = 0
                for dc in range(NDC):
                    sl = slice(dc * P, (dc + 1) * P)
                    dma_eng[si % 2].dma_start(wq_t[dc][:, 0:256],
                                              wqt[sl, 0:256])
                    si += 1
                    nchunk = 4 if dc < 3 else 2
                    cw = S // nchunk
                    for hf in range(nchunk):
                        dma_eng[si % 2].dma_start(
                            xt_t[dc][:, hf * cw:(hf + 1) * cw],
                            xt[sl, hf * cw:(hf + 1) * cw])
                        si += 1

                # Q: e-tile PAIRS concurrently — during the x-DMA ramp each
                # arriving chunk feeds 8 matmuls instead of 4, keeping PE busy
                for pair in range(2):
                    ets = (2 * pair, 2 * pair + 1)
                    ps = {et: qk_psums(et) for et in ets}
                    for dc in range(NDC):
                        for et in ets:
                            lhsT = wq_t[dc][:, et * P:(et + 1) * P]
                            for sb_ in range(NSB):
                                nc.tensor.matmul(
                                    ps[et][2][sb_], lhsT,
                                    xt_t[dc][:, sb_ * 512:(sb_ + 1) * 512],
                                    start=(dc == 0), stop=(dc == NDC - 1))
                    for et in ets:
                        qk_evac(qt_t, et, ps[et][0], ps[et][1], bq_sb)
                    if pair == 0:
                        for dc in range(NDC):
                            sl = slice(dc * P, (dc + 1) * P)
                            dma_eng[dc % 2].dma_start(
                                wq_t[dc][:, 256:E], wqt[sl, 256:E])

            with tc.tile_pool(name="pk", bufs=1) as pk:
                wk_t = [pk.tile([P, E], BF, name=f"wk{dc}", tag=f"wk{dc}") for dc in range(NDC)]
                for dc in range(NDC):
                    sl = slice(dc * P, (dc + 1) * P)
                    dma_eng[dc % 2].dma_start(wk_t[dc][:], wkt[sl, :])

                # K: sequential e-tile groups (input already resident)
                for et in range(HPC):
                    psA, psB, outs = qk_psums(et)
                    for dc in range(NDC):
                        lhsT = wk_t[dc][:, et * P:(et + 1) * P]
                        for sb_ in range(NSB):
                            nc.tensor.matmul(
                                outs[sb_], lhsT,
                                xt_t[dc][:, sb_ * 512:(sb_ + 1) * 512],
                                start=(dc == 0), stop=(dc == NDC - 1))
                    qk_evac(kt_t, et, psA, psB, bk_sb)

            for dc in range(NDC):
                sl = slice(dc * P, (dc + 1) * P)
                dma_eng[dc % 2].dma_start(wv_t[dc][:], wvt[sl, :])

            # i-block 0+1 scores + exps: their ACT work overlaps the V
            # matmuls, taking ~21us of exp off the phase-2 critical path
            for hp in range(2):
                group_etiles[(0, hp)] = sc_group(0, hp, lambda jt, hp=hp: e0_t[hp][jt])
            for hp in range(2):
                group_etiles[(1, hp)] = sc_group(1, hp, lambda jt, hp=hp: e1_t[hp][jt])

            # V: out[s_tile(128), e(512)] accumulated over d-chunks
            for st in range(NST):
                psv = pst.tile([P, 512], F32, name="psv", tag="acc", bufs=4)
                for dc in range(NDC):
                    nc.tensor.matmul(
                        psv[:], xt_t[dc][:, st * P:(st + 1) * P], wv_t[dc][:],
                        start=(dc == 0), stop=(dc == NDC - 1))
                nc.vector.tensor_copy(
                    v_t[st][:, :, 0:DK],
                    psv[:].rearrange("p (h w) -> p h w", h=HPC))

        # ------------------------------------------------------------------
        # Phase 2+3, software-pipelined one group deep
        # ------------------------------------------------------------------
        with tc.tile_pool(name="p2", bufs=1) as p2, \
                tc.tile_pool(name="exps", bufs=26) as epool, \
                tc.tile_pool(name="small", bufs=8) as spool, \
                tc.tile_pool(name="yout", bufs=4) as ypool:
            o_t = [p2.tile([P, HPC * DK], BF, name=f"o{st}", tag=f"o{st}")
                   for st in range(NST)]
            wot_t = [p2.tile([P, D], BF, name=f"wot{ec}", tag=f"wot{ec}")
                     for ec in range(HPC)]
            ot_t = [p2.tile([P, S], BF, name=f"ot{ec}", tag=f"ot{ec}")
                    for ec in range(HPC)]
            for ec in range(HPC):
                nc.sync.dma_start(wot_t[ec][:], wot[ec * P:(ec + 1) * P, :])

            def av_one(ib, h, k, t, etiles):
                it = 4 * ib + t
                po = pst.tile([P, 512], F32, name="po", tag="acc", bufs=4)
                for jt in range(it + 1):
                    nc.tensor.matmul(
                        po[:, 0:DKP],
                        etiles[jt][:, k * 512 + t * P:k * 512 + (t + 1) * P],
                        v_t[jt][:, h, :],
                        start=(jt == 0), stop=(jt == it))
                rec = spool.tile([P, 1], F32, name="rec", tag="rec")
                nc.vector.reciprocal(rec[:], po[:, DK:DKP])
                nc.vector.tensor_scalar_mul(
                    o_t[it][:, h * P:(h + 1) * P], po[:, 0:DK], rec[:])

            def tp_p3_one(st):
                for ec in range(HPC):
                    # transpose via regular matmul against the identity
                    # (pipelines at stream rate, unlike PE transpose-mode)
                    pt = pst.tile([P, P], F32, name="pt", tag="acc", bufs=4)
                    nc.tensor.matmul(
                        pt[:], o_t[st][:, ec * P:(ec + 1) * P], ident[:],
                        start=True, stop=True)
                    nc.vector.tensor_copy(
                        ot_t[ec][:, st * P:(st + 1) * P], pt[:])
                for ob in range(NSB):
                    py = pst.tile([P, 512], F32, name="py", tag="acc", bufs=4)
                    for ec in range(HPC):
                        nc.tensor.matmul(
                            py[:], ot_t[ec][:, st * P:(st + 1) * P],
                            wot_t[ec][:, ob * 512:(ob + 1) * 512],
                            start=(ec == 0), stop=(ec == HPC - 1))
                    y = ypool.tile([P, 512], BF, name="y", tag="y")
                    # y casts alternate DVE/ACT: the ~690ns PSUM->bf16 casts
                    # gate the shared acc-PSUM ring, and ACT has slack here
                    # (i-block 0/1 exps moved to P1)
                    if ob % 2 == 0:
                        nc.vector.tensor_copy(y[:], py[:])
                    else:
                        nc.scalar.activation(
                            y[:], py[:], mybir.ActivationFunctionType.Copy)
                    nc.sync.dma_start(
                        outd[st * P:(st + 1) * P,
                             ob * 512:(ob + 1) * 512],
                        y[:])

            def av_group(ib, hp):
                ets = group_etiles[(ib, hp)]
                if hp == 0:
                    for k in range(2):
                        for t in range(4):
                            av_one(ib, 2 * hp + k, k, t, ets)
                else:
                    # transposes trail their s-tile's AVs by one t: the
                    # transpose LDW needs the DVE evac of its s-tile's last
                    # AV; the lag gives that chain slack and spreads the
                    # DVE evacuation load
                    for t in range(4):
                        for k in range(2):
                            av_one(ib, 2 * hp + k, k, t, ets)
                        if t >= 1:
                            tp_p3_one(4 * ib + t - 1)
                    tp_p3_one(4 * ib + 3)

            def et_ring(jt):
                return epool.tile([P, 1024], BF, name="et", tag="exp")

            # slot g emits scores+exp for group g, then AV/P3 for group g-1:
            # the scheduler uses the prior group's (fully-exp'd) work as PE
            # filler while this group's scores wait on ACT.  Groups (0,*)
            # and (1,*) had scores+exps in P1, so their AV work opens the
            # reservoir.
            av_group(0, 0)
            av_group(0, 1)
            slots = [(2, 0), (2, 1), (3, 0), (3, 1)]
            prevs = [(1, 0), (1, 1), (2, 0), (2, 1)]
            for g, pg in zip(slots, prevs):
                group_etiles[g] = sc_group(g[0], g[1], et_ring)
                av_group(*pg)
            av_group(3, 0)
            av_group(3, 1)

    nc.finalize()
    return nc


_NC_CACHE = {}


def _get_nc():
    if "nc" not in _NC_CACHE:
        _NC_CACHE["nc"] = _build_nc()
    return _NC_CACHE["nc"]


def _make_in_maps(x, wq, bq, wk, bk, wv, wo):
    scale = np.float32(1.0 / np.sqrt(DK))
    in_maps = []
    for c in range(NCORES):
        b = c // 4
        g = c % 4
        sl = slice(E * g, E * (g + 1))
        in_maps.append({
            "xt": np.ascontiguousarray(x[b].T).astype(BF16),
            "wqt": np.ascontiguousarray((wq[sl] * scale).T).astype(BF16),
            "wkt": np.ascontiguousarray(wk[sl].T).astype(BF16),
            "wvt": np.ascontiguousarray(wv[sl].T).astype(BF16),
            "wot": np.ascontiguousarray(wo[:, sl].T).astype(BF16),
            "bq": (bq[sl] * scale).astype(np.float32),
            "bk": bk[sl].astype(np.float32),
        })
    return in_maps


def _assemble(core_outs, wv_bias_vec):
    out = np.empty((B, S, D), np.float32)
    for b in range(B):
        acc = core_outs[4 * b].astype(np.float32)
        for g in range(1, 4):
            acc = acc + core_outs[4 * b + g].astype(np.float32)
        out[b] = acc + wv_bias_vec
    return out


def kernel(x, wq, bq, wk, bk, wv, bv, wo, bo, mask, _trace=False):
    x = np.asarray(x, dtype=np.float32)
    wq = np.asarray(wq, dtype=np.float32)
    bq = np.asarray(bq, dtype=np.float32)
    wk = np.asarray(wk, dtype=np.float32)
    bk = np.asarray(bk, dtype=np.float32)
    wv = np.asarray(wv, dtype=np.float32)
    bv = np.asarray(bv, dtype=np.float32)
    wo = np.asarray(wo, dtype=np.float32)
    bo = np.asarray(bo, dtype=np.float32)

    in_maps = _make_in_maps(x, wq, bq, wk, bk, wv, wo)
    nc = _get_nc()
    res = run_bass_kernel_spmd(nc, in_maps, core_ids=list(range(NCORES)),
                               trace=_trace)
    core_outs = [res.results[c]["out"] for c in range(NCORES)]
    # rows of softmax sum to 1 -> per-head V-bias contributes wo[:, sl] @ bv
    bias_vec = (bo + wo @ bv).astype(np.float32)
    out = _assemble(core_outs, bias_vec)
    if _trace:
        return out, res
    return out

